# revision 1
# baseline (speedup 1.0000x reference)
"""Trainium2 Bass kernel for nn_AttentionBlock (B=8, L=2048, E=512, FF=2048).

Strategy: data-parallel over batch — core b computes batch item b end-to-end
(no collectives). All activations live transposed ([feature, token], feature on
partitions) so every matmul contracts over the partition dim with natural
layouts and no on-device transposes:

  phase A: kT = Wk^T-proj, v = x@Wv^T (natural layout)
  phase B (per 512-token l-chunk): qT proj, scoresT[s,l] on PE, exp on ACT (no
    max-subtraction — scores bounded ~12), colsum via ones-matmul (output
    arrives broadcast over partitions), 1/colsum on the DVE (fast approx
    reciprocal), AV matmul, y = x + attn (residual) in place. Chunk 0's LN1 is
    folded into chunk 1's attention stream so phase C can start on the FFN
    immediately.
  phase C (per l-chunk, software-pipelined across chunks): LN1 via ones-matmul
    partition sums + ACT sqrt + DVE fast reciprocal, FFN1+relu(+b1), FFN2,
    residual, LN2, store outT. LN work for neighbouring chunks is interleaved
    into the FFN matmul stream so the PE never idles on DVE/ACT epilogues.

Precision: matmuls run in float32r (fp32 with 11-bit mantissa, 4x faster than
plain fp32 on the PE); inputs are pre-rounded on the host and on-device
producers write f32r tiles so the PE consumes correctly-rounded data. The
accuracy-insensitive operands (post-softmax weights, V, the relu activations
and W2) are fp16 instead — same matmul speed, half the SBUF/DMA cost, and the
exp carries a 2^-4 bias so fp16 cannot overflow (it cancels exactly in the
softmax ratio).

ACT table sets: only Exp/Copy (exp_and_others) and Sqrt/Square/Relu/Copy
(sqrt_and_others) are used — two table loads total.
"""
import math
from contextlib import ExitStack

import numpy as np

import concourse.bass as bass
import concourse.bacc as bacc
import concourse.tile as tile
from concourse import mybir
from concourse.bass_utils import run_bass_kernel_spmd

P = 128
B, L, E, FF = 8, 2048, 512, 2048
NDOM = 32
EPS = 1e-5
SCALE = (1.0 / math.sqrt(E)) * 2.0 * math.log(NDOM)
EXPB = -4.0 * math.log(2.0)   # exp(s+EXPB)=exp(s)/16, cancels in softmax

EO = E // P           # 4  e-chunks
FO = FF // P          # 16 f-chunks
LC = 512              # l-chunk (matmul free dim)
NLC = L // LC         # 4  l-chunks
SB = L // P           # 16 s-blocks

F32 = mybir.dt.float32
F32R = mybir.dt.float32r
F16 = mybir.dt.float16
AF = mybir.ActivationFunctionType
OP = mybir.AluOpType

_TRACE = False
LAST_RESULT = None
_CACHE = {}


def _round_fp32r(x):
    """Round-to-nearest-even fp32 -> fp32r (low 12 mantissa bits cleared)."""
    u = np.ascontiguousarray(x, dtype=np.float32).view(np.uint32)
    frac = u & np.uint32(0xFFF)
    base = u & np.uint32(0xFFFFF000)
    up = (frac > 0x800) | ((frac == 0x800) & (((u >> 12) & 1) == 1))
    return (base + np.where(up, np.uint32(0x1000), np.uint32(0))).view(np.float32)


def _build(ln1_trivial, ln2_trivial, b2_zero):
    nc = bacc.Bacc("TRN2", debug=False, target_bir_lowering=False, num_devices=B)

    xt_d = nc.dram_tensor("xt", [E, L], F32R, kind="ExternalInput")
    wqt_d = nc.dram_tensor("wqt", [E, E], F32R, kind="ExternalInput")
    wkt_d = nc.dram_tensor("wkt", [E, E], F32R, kind="ExternalInput")
    wvt_d = nc.dram_tensor("wvt", [E, E], F32R, kind="ExternalInput")
    w1t_d = nc.dram_tensor("w1t", [E, FF], F32R, kind="ExternalInput")
    w2t_d = nc.dram_tensor("w2t", [FF, E], F16, kind="ExternalInput")
    b1_d = nc.dram_tensor("b1v", [FF], F32, kind="ExternalInput")
    b2_d = None if b2_zero else nc.dram_tensor("b2v", [E], F32, kind="ExternalInput")
    ln1w_d = ln1b_d = ln2w_d = ln2b_d = None
    if not ln1_trivial:
        ln1w_d = nc.dram_tensor("ln1w", [E], F32, kind="ExternalInput")
        ln1b_d = nc.dram_tensor("ln1b", [E], F32, kind="ExternalInput")
    if not ln2_trivial:
        ln2w_d = nc.dram_tensor("ln2w", [E], F32, kind="ExternalInput")
        ln2b_d = nc.dram_tensor("ln2b", [E], F32, kind="ExternalInput")
    out_d = nc.dram_tensor("outt", [E, L], F32, kind="ExternalOutput")

    xt_r = xt_d.ap().rearrange("(eo p) l -> p eo l", p=P)
    wqt_r = wqt_d.ap().rearrange("(eo p) f -> p eo f", p=P)
    wkt_r = wkt_d.ap().rearrange("(eo p) f -> p eo f", p=P)
    wvt_r = wvt_d.ap().rearrange("(eo p) f -> p eo f", p=P)
    w1t_r = w1t_d.ap().rearrange("(eo p) f -> p eo f", p=P)
    w2t_r = w2t_d.ap().rearrange("(fo p) e -> p fo e", p=P)
    out_r = out_d.ap().rearrange("(eo p) l -> p eo l", p=P)

    with tile.TileContext(nc) as tc, ExitStack() as stk:
        const = stk.enter_context(tc.tile_pool(name="const", bufs=1))
        px = stk.enter_context(tc.tile_pool(name="px", bufs=1))
        pstat = stk.enter_context(tc.tile_pool(name="pstat", bufs=1))
        ph = stk.enter_context(tc.tile_pool(name="ph", bufs=2))
        pysq = stk.enter_context(tc.tile_pool(name="pysq", bufs=1))
        pwearly = stk.enter_context(tc.tile_pool(name="pwearly", bufs=1))

        ones_f = const.tile([P, P], F32)
        ones_r = const.tile([P, P], F32R)
        ones_h = const.tile([P, P], F16)
        eps_t = const.tile([P, 1], F32)
        expb_t = const.tile([P, 1], F32)
        b1_t = const.tile([P, FO], F32)
        nc.vector.memset(ones_f[:], 1.0)
        nc.vector.tensor_copy(ones_r[:], ones_f[:])
        nc.vector.tensor_copy(ones_h[:], ones_f[:])
        nc.vector.memset(eps_t[:], EPS)
        nc.vector.memset(expb_t[:], EXPB)
        b1_r = b1_d.ap().rearrange("(fo p) -> p fo", p=P)
        b2_t = None
        if b2_d is not None:
            b2_t = const.tile([P, EO], F32)
            nc.sync.dma_start(b2_t[:], b2_d.ap().rearrange("(eo p) -> p eo", p=P))
        ln1w_t = ln1b_t = ln2w_t = ln2b_t = None
        if ln1w_d is not None:
            ln1w_t = const.tile([P, EO], F32)
            ln1b_t = const.tile([P, EO], F32)
            nc.sync.dma_start(ln1w_t[:], ln1w_d.ap().rearrange("(eo p) -> p eo", p=P))
            nc.sync.dma_start(ln1b_t[:], ln1b_d.ap().rearrange("(eo p) -> p eo", p=P))
        if ln2w_d is not None:
            ln2w_t = const.tile([P, EO], F32)
            ln2b_t = const.tile([P, EO], F32)
            nc.sync.dma_start(ln2w_t[:], ln2w_d.ap().rearrange("(eo p) -> p eo", p=P))
            nc.sync.dma_start(ln2b_t[:], ln2b_d.ap().rearrange("(eo p) -> p eo", p=P))

        xt = px.tile([P, EO, L], F32R)          # x^T, becomes y = x + attn in B
        state = {}

        def ln_stats_rest(i, tag, s_ps, s2_ps, y_sl):
            """negmean/meansq on ACT, var+rstd via sqrt + fast reciprocal."""
            negmean = pstat.tile([P, LC], F32, tag="nm", name=f"nm{tag}_{i}")
            msq = pstat.tile([P, LC], F32, tag="msq", name=f"msq{tag}_{i}")
            ex2 = pstat.tile([P, LC], F32, tag="ex2", name=f"ex2{tag}_{i}")
            rstd = pstat.tile([P, LC], F32, tag="rstd", name=f"rstd{tag}_{i}")
            nc.scalar.activation(negmean[:], s_ps[:], AF.Copy, scale=-1.0 / E)
            nc.scalar.activation(msq[:], s_ps[:], AF.Square, scale=1.0 / E)
            nc.vector.tensor_scalar_mul(ex2[:], s2_ps[:], 1.0 / E)
            nc.vector.tensor_tensor(ex2[:], ex2[:], msq[:], OP.subtract)
            nc.scalar.activation(ex2[:], ex2[:], AF.Sqrt, bias=eps_t[:])
            nc.vector.reciprocal_approx_fast(rstd[:], ex2[:])
            return y_sl, negmean, rstd

        def ln1_apply(i):
            y_sl, negmean, rstd = state.pop(("ln1", i))
            h = ph.tile([P, EO, LC], F32R, tag="h", name=f"h{i}")
            for ec in range(EO):
                t = pstat.tile([P, LC], F32, tag="lnapp", name=f"la1_{i}_{ec}")
                nc.vector.tensor_tensor(t[:], y_sl[ec].bitcast(F32),
                                        negmean[:], OP.add)
                if ln1_trivial:
                    nc.vector.tensor_tensor(h[:, ec, :], t[:], rstd[:], OP.mult)
                else:
                    nc.vector.tensor_tensor(t[:], t[:], rstd[:], OP.mult)
                    nc.scalar.activation(h[:, ec, :], t[:], AF.Identity,
                                         bias=ln1b_t[:, ec:ec + 1],
                                         scale=ln1w_t[:, ec:ec + 1])
            state[("h", i)] = h

        with tc.tile_pool(name="pkv", bufs=1) as pkv, \
             tc.tile_pool(name="pwq", bufs=1) as pwq, \
             tc.tile_pool(name="psMM", bufs=2, space="PSUM") as psMM:
            wq = pwq.tile([P, EO, E], F32R)
            kt = pkv.tile([P, EO, L], F32R)     # k^T [e, s]
            vt = pkv.tile([P, SB, E], F16)      # v natural [s, e]

            # ---------------- phase A: kT, v projections ----------------
            with tc.tile_pool(name="pwk", bufs=1) as pwk:
                wk = pwk.tile([P, EO, E], F32R)
                wv = pwk.tile([P, EO, E], F32R)
                # first-need DMA order; xt on the gpsimd queue in parallel
                def load_x(lc, engs=(nc.scalar, nc.gpsimd)):
                    ls = lc * LC
                    for eo in range(EO):
                        engs[eo % len(engs)].dma_start(
                            xt[:, eo, ls:ls + LC], xt_r[:, eo, ls:ls + LC])

                # first-need tensors (wk + x chunk 0) split across all three
                # DMA rings; each ring drains in order at ~1/3 aggregate BW
                nc.sync.dma_start(wk[:, 0, :], wkt_r[:, 0, :])
                nc.sync.dma_start(wk[:, 1, :], wkt_r[:, 1, :])
                nc.scalar.dma_start(xt[:, 0, 0:LC], xt_r[:, 0, 0:LC])
                nc.gpsimd.dma_start(xt[:, 2, 0:LC], xt_r[:, 2, 0:LC])
                nc.scalar.dma_start(xt[:, 1, 0:LC], xt_r[:, 1, 0:LC])
                nc.gpsimd.dma_start(xt[:, 3, 0:LC], xt_r[:, 3, 0:LC])
                nc.scalar.dma_start(wk[:, 2, :], wkt_r[:, 2, :])
                nc.gpsimd.dma_start(wk[:, 3, :], wkt_r[:, 3, :])
                nc.scalar.dma_start(wv[:, 0, :], wvt_r[:, 0, :])
                nc.gpsimd.dma_start(wv[:, 1, :], wvt_r[:, 1, :])
                nc.scalar.dma_start(wv[:, 2, :], wvt_r[:, 2, :])
                nc.gpsimd.dma_start(wv[:, 3, :], wvt_r[:, 3, :])
                load_x(1)
                nc.sync.dma_start(wq[:], wqt_r)
                load_x(2)
                load_x(3)

                for lc in range(NLC):
                    ls = lc * LC
                    for fb in range(EO):
                        kp = psMM.tile([P, LC], F32, tag="mm")
                        for kk in range(EO):
                            nc.tensor.matmul(kp[:], wk[:, kk, fb * P:(fb + 1) * P],
                                             xt[:, kk, ls:ls + LC],
                                             start=(kk == 0), stop=(kk == EO - 1))
                        nc.scalar.activation(kt[:, fb, ls:ls + LC], kp[:], AF.Copy)
                    for j in range(4):
                        lb = lc * 4 + j
                        vp = psMM.tile([P, LC], F32, tag="mm")
                        for kk in range(EO):
                            nc.tensor.matmul(
                                vp[:], xt[:, kk, lb * P:(lb + 1) * P],
                                wv[:, kk, :], start=(kk == 0), stop=(kk == EO - 1))
                        nc.scalar.activation(vt[:, lb, :], vp[:], AF.Copy)

            # ---------------- phase B: attention per l-chunk ----------------
            with (
                tc.tile_pool(name="pq", bufs=1) as pq,
                tc.tile_pool(name="pp", bufs=2) as pp,
                tc.tile_pool(name="paon", bufs=1) as paon,
                tc.tile_pool(name="psAO", bufs=4, space="PSUM") as psAO,
                tc.tile_pool(name="psCS", bufs=1, space="PSUM") as psCS,
                tc.tile_pool(name="psSB", bufs=1, space="PSUM") as psSB,
            ):
                # first FFN weight slices land here during phase B (the
                # main w1t/w2t pools reuse B's SBUF and can only fill after
                # B drains)
                w1e = pwearly.tile([P, EO, 4 * P], F32R)
                w2e = pwearly.tile([P, 2, E], F16)
                for fo in range(4):
                    nc.scalar.dma_start(w1e[:, :, fo * P:(fo + 1) * P],
                                        w1t_r[:, :, fo * P:(fo + 1) * P])
                    if fo < 2:
                        nc.gpsimd.dma_start(w2e[:, fo, :], w2t_r[:, fo, :])

                def q_proj(lc):
                    ls = lc * LC
                    qt = pq.tile([P, EO, LC], F32R, tag="q", name=f"qt{lc}")
                    for fb in range(EO):
                        qp = psMM.tile([P, LC], F32, tag="mm", name=f"qp{lc}_{fb}")
                        for kk in range(EO):
                            nc.tensor.matmul(
                                qp[:], wq[:, kk, fb * P:(fb + 1) * P],
                                xt[:, kk, ls:ls + LC],
                                start=(kk == 0), stop=(kk == EO - 1))
                        nc.scalar.activation(qt[:, fb, :], qp[:], AF.Copy)
                    return qt

                # chunk-0 LN1 pieces, injected into chunk 1's attention stream
                def ln1c0_sq():
                    y_sl = [xt[:, ec, 0:LC] for ec in range(EO)]
                    ysq = pysq.tile([P, EO, LC], F32R, tag="ysq", name="ysq1_0")
                    for ec in range(EO):
                        nc.scalar.activation(ysq[:, ec, :], y_sl[ec].bitcast(F32),
                                             AF.Square)
                    state["c0"] = (y_sl, ysq)

                def ln1c0_sum1():
                    y_sl, ysq = state["c0"]
                    s_ps = psSB.tile([P, LC], F32, tag="sums", name="s1_0")
                    for ec in range(EO):
                        nc.tensor.matmul(s_ps[:], ones_r[:], y_sl[ec],
                                         start=(ec == 0), stop=(ec == EO - 1))
                    negmean = pstat.tile([P, LC], F32, tag="nm", name="nm1_0")
                    nc.scalar.activation(negmean[:], s_ps[:], AF.Copy,
                                         scale=-1.0 / E)
                    msq = pstat.tile([P, LC], F32, tag="msq", name="msq1_0")
                    nc.scalar.activation(msq[:], s_ps[:], AF.Square, scale=1.0 / E)
                    state["c0b"] = (negmean, msq)

                def ln1c0_sum2():
                    y_sl, ysq = state.pop("c0")
                    negmean, msq = state.pop("c0b")
                    s2_ps = psSB.tile([P, LC], F32, tag="sums", name="s2_0")
                    for ec in range(EO):
                        nc.tensor.matmul(s2_ps[:], ones_r[:], ysq[:, ec, :],
                                         start=(ec == 0), stop=(ec == EO - 1))
                    ex2 = pstat.tile([P, LC], F32, tag="ex2", name="ex21_0")
                    rstd = pstat.tile([P, LC], F32, tag="rstd", name="rstd1_0")
                    nc.vector.tensor_scalar_mul(ex2[:], s2_ps[:], 1.0 / E)
                    nc.vector.tensor_tensor(ex2[:], ex2[:], msq[:], OP.subtract)
                    nc.scalar.activation(ex2[:], ex2[:], AF.Sqrt, bias=eps_t[:])
                    nc.vector.reciprocal_approx_fast(rstd[:], ex2[:])
                    state[("ln1", 0)] = ([xt[:, ec, 0:LC] for ec in range(EO)],
                                         negmean, rstd)

                qts = {0: q_proj(0)}
                for lc in range(NLC):
                    ls = lc * LC
                    qt = qts.pop(lc)
                    pexp = pp.tile([P, SB, LC], F16, tag="pexp", name=f"pexp{lc}")
                    ao = [psAO.tile([P, LC], F32, tag="ao", name=f"ao{lc}_{e}")
                          for e in range(EO)]
                    cs = psCS.tile([P, LC], F32, tag="cs", name=f"cs{lc}")

                    inject = {}
                    if lc == 1:
                        inject = {4: ln1c0_sq, 7: ln1c0_sum1, 10: ln1c0_sum2,
                                  13: lambda: ln1_apply(0)}

                    st_ps = []

                    def scores(sb, qt=qt, st_ps=st_ps, lc=lc):
                        sp = psMM.tile([P, LC], F32, tag="mm",
                                       name=f"sp{lc}_{sb}")
                        for kk in range(EO):
                            nc.tensor.matmul(
                                sp[:], kt[:, kk, sb * P:(sb + 1) * P],
                                qt[:, kk, :],
                                start=(kk == 0), stop=(kk == EO - 1))
                        st_ps.append(sp)

                    def expevict(sb, pexp=pexp, st_ps=st_ps):
                        nc.scalar.activation(pexp[:, sb, :], st_ps[sb][:], AF.Exp,
                                             bias=expb_t[:])

                    def av(sb, pexp=pexp, ao=ao, cs=cs):
                        nc.tensor.matmul(cs[:], ones_h[:], pexp[:, sb, :],
                                         start=(sb == 0), stop=(sb == SB - 1))
                        for eb in range(EO):
                            nc.tensor.matmul(
                                ao[eb][:], vt[:, sb, eb * P:(eb + 1) * P],
                                pexp[:, sb, :],
                                start=(sb == 0), stop=(sb == SB - 1))

                    scores(0)
                    expevict(0)
                    for sb in range(1, SB):
                        scores(sb)
                        expevict(sb)
                        av(sb - 1)
                        if sb in inject:
                            inject[sb]()
                    av(SB - 1)

                    # next chunk's q projection keeps the PE busy while the
                    # DVE normalizes + finishes this chunk's residual
                    if lc + 1 < NLC:
                        qts[lc + 1] = q_proj(lc + 1)

                    rcs = pstat.tile([P, LC], F32, tag="rcs")
                    nc.vector.reciprocal_approx_fast(rcs[:], cs[:])
                    # y = x + ao * rcs   (in place into xt)
                    for ec in range(EO):
                        aon = paon.tile([P, LC], F32, tag="aon")
                        nc.vector.tensor_tensor(aon[:], ao[ec][:], rcs[:], OP.mult)
                        nc.vector.tensor_tensor(
                            xt[:, ec, ls:ls + LC],
                            xt[:, ec, ls:ls + LC].bitcast(F32), aon[:], OP.add)

                state["wearly"] = (w1e, w2e)

        # ---------------- phase C: LN1, FFN, LN2 per l-chunk ----------------
        with (
            tc.tile_pool(name="pw1", bufs=1) as pw1,
            tc.tile_pool(name="pw2", bufs=1) as pw2,
            tc.tile_pool(name="py2", bufs=1) as py2,
            tc.tile_pool(name="prelu", bufs=1) as prelu,
            tc.tile_pool(name="pout", bufs=1) as pout,
            tc.tile_pool(name="psF1", bufs=2, space="PSUM") as psF1,
            tc.tile_pool(name="psF2", bufs=4, space="PSUM") as psF2,
            tc.tile_pool(name="psS", bufs=2, space="PSUM") as psS,
        ):
            w1t = pw1.tile([P, EO, FF], F32R)
            w2t = pw2.tile([P, FO, E], F16)
            nc.scalar.dma_start(b1_t[:], b1_r)
            w1e, w2e = state.pop("wearly")
            # sliced loads so FFN matmuls start as soon as each slice lands
            # (slices 0-3 were staged into w1e/w2e during phase B)
            for g in range(1, EO):
                nc.sync.dma_start(w1t[:, :, g * 4 * P:(g + 1) * 4 * P],
                                  w1t_r[:, :, g * 4 * P:(g + 1) * 4 * P])
            for fo in range(2, FO):
                nc.gpsimd.dma_start(w2t[:, fo, :], w2t_r[:, fo, :])

            def w1_sl(fo):
                if fo < 4:
                    return w1e[:, :, fo * P:(fo + 1) * P]
                return w1t[:, :, fo * P:(fo + 1) * P]

            def w2_sl(fo):
                if fo < 2:
                    return w2e[:, fo, :]
                return w2t[:, fo, :]

            def ln_pre(i):
                """squares + partition sums + rstd for chunk i's LN1."""
                ls = i * LC
                y_sl = [xt[:, ec, ls:ls + LC] for ec in range(EO)]
                ysq = pysq.tile([P, EO, LC], F32R, tag="ysq", name=f"ysq1_{i}")
                for ec in range(EO):
                    nc.scalar.activation(ysq[:, ec, :], y_sl[ec].bitcast(F32),
                                         AF.Square)
                s_ps = psS.tile([P, LC], F32, tag="sums", name=f"s1_{i}")
                s2_ps = psS.tile([P, LC], F32, tag="sums", name=f"s2_{i}")
                for ec in range(EO):
                    nc.tensor.matmul(s_ps[:], ones_r[:], y_sl[ec],
                                     start=(ec == 0), stop=(ec == EO - 1))
                for ec in range(EO):
                    nc.tensor.matmul(s2_ps[:], ones_r[:], ysq[:, ec, :],
                                     start=(ec == 0), stop=(ec == EO - 1))
                state[("ln1", i)] = ln_stats_rest(i, "1", s_ps, s2_ps, y_sl)

            def ffn_start(i):
                relu1 = prelu.tile([P, FO, LC], F16, tag="relu1",
                                   name=f"relu1_{i}")
                ao2 = [psF2.tile([P, LC], F32, tag="ao2", name=f"ao2_{i}_{e}")
                       for e in range(EO)]
                state[("ffn", i)] = (relu1, ao2)

            def ffn1(i, fo):
                relu1, _ = state[("ffn", i)]
                h = state[("h", i)]
                fp = psF1.tile([P, LC], F32, tag="f1", name=f"fp{i}_{fo}")
                w1s = w1_sl(fo)
                for kk in range(EO):
                    nc.tensor.matmul(fp[:], w1s[:, kk, :], h[:, kk, :],
                                     start=(kk == 0), stop=(kk == EO - 1))
                nc.scalar.activation(relu1[:, fo, :], fp[:], AF.Relu,
                                     bias=b1_t[:, fo:fo + 1])

            def ffn2(i, fo):
                relu1, ao2 = state[("ffn", i)]
                w2s = w2_sl(fo)
                for eb in range(EO):
                    nc.tensor.matmul(
                        ao2[eb][:], w2s[:, eb * P:(eb + 1) * P],
                        relu1[:, fo, :],
                        start=(fo == 0), stop=(fo == FO - 1))

            def resid2(i):
                """z = ffn + h (+b2), squares + partition sums, per-ec
                interleaved so the post-FFN critical path is short."""
                _, ao2 = state[("ffn", i)]
                h = state.pop(("h", i))
                if b2_t is not None:
                    for ec in range(EO):
                        nc.vector.tensor_tensor(
                            h[:, ec, :], h[:, ec, :].bitcast(F32),
                            b2_t[:, ec:ec + 1].to_broadcast((P, LC)), OP.add)
                y2 = py2.tile([P, EO, LC], F32R, tag="y2", name=f"y2_{i}")
                ysq = pysq.tile([P, EO, LC], F32R, tag="ysq", name=f"ysq2_{i}")
                s_ps = psS.tile([P, LC], F32, tag="sums", name=f"s3_{i}")
                s2_ps = psS.tile([P, LC], F32, tag="sums", name=f"s4_{i}")
                for ec in range(EO):
                    nc.vector.tensor_tensor(y2[:, ec, :], ao2[ec][:],
                                            h[:, ec, :].bitcast(F32), OP.add)
                    nc.scalar.activation(ysq[:, ec, :], y2[:, ec, :].bitcast(F32),
                                         AF.Square)
                    nc.tensor.matmul(s_ps[:], ones_r[:], y2[:, ec, :],
                                     start=(ec == 0), stop=(ec == EO - 1))
                    nc.tensor.matmul(s2_ps[:], ones_r[:], ysq[:, ec, :],
                                     start=(ec == 0), stop=(ec == EO - 1))
                state.pop(("ffn", i))
                state[("y2", i)] = (y2, s_ps, s2_ps)

            def ln2_full(i):
                y2, s_ps, s2_ps = state.pop(("y2", i))
                y2_sl = [y2[:, ec, :] for ec in range(EO)]
                _, negmean, rstd = ln_stats_rest(i, "2", s_ps, s2_ps, y2_sl)
                ls = i * LC
                outt = pout.tile([P, EO, LC], F32, tag="out", name=f"out{i}")
                # subtracts only need negmean — they hide under the ACT sqrt
                # and DVE reciprocal that produce rstd
                ts = []
                for ec in range(EO):
                    t = pstat.tile([P, LC], F32, tag=f"lnapp{ec}",
                                   name=f"la2_{i}_{ec}")
                    nc.vector.tensor_tensor(t[:], y2_sl[ec].bitcast(F32),
                                            negmean[:], OP.add)
                    ts.append(t)
                for ec in range(EO):
                    t = ts[ec]
                    if ln2_trivial:
                        nc.vector.tensor_tensor(outt[:, ec, :], t[:], rstd[:],
                                                OP.mult)
                    else:
                        nc.vector.tensor_tensor(t[:], t[:], rstd[:], OP.mult)
                        nc.scalar.activation(outt[:, ec, :], t[:], AF.Identity,
                                             bias=ln2b_t[:, ec:ec + 1],
                                             scale=ln2w_t[:, ec:ec + 1])
                    nc.sync.dma_start(out_r[:, ec, ls:ls + LC], outt[:, ec, :])

            # ---- pipelined emission (h(0) already computed in phase B) ----
            for i in range(NLC):
                ffn_start(i)
                ffn1(i, 0)
                ffn1(i, 1)
                if i > 0:
                    ln2_full(i - 1)          # PE: 8 stats MMs amid FFN stream
                ffn2(i, 0)
                ffn1(i, 2)
                ffn2(i, 1)
                ffn1(i, 3)
                ffn2(i, 2)
                if i + 1 < NLC:
                    ln_pre(i + 1)            # next chunk's LN1 stats
                ffn1(i, 4)
                ffn2(i, 3)
                ffn1(i, 5)
                ffn2(i, 4)
                ffn1(i, 6)
                ffn2(i, 5)
                if i + 1 < NLC:
                    ln1_apply(i + 1)         # h(i+1) ready before FFN(i) ends
                for fo in range(7, FO):
                    ffn1(i, fo)
                    ffn2(i, fo - 1)
                ffn2(i, FO - 1)
                resid2(i)
            ln2_full(NLC - 1)

    nc.compile()
    return nc


def kernel(x, in_proj_w, ln1_w, ln1_b, ln2_w, ln2_b, w1, b1, w2, b2):
    global LAST_RESULT
    x = np.asarray(x, dtype=np.float32)
    in_proj_w = np.asarray(in_proj_w, dtype=np.float32)
    w1 = np.asarray(w1, dtype=np.float32)
    w2 = np.asarray(w2, dtype=np.float32)
    b1 = np.asarray(b1, dtype=np.float32)
    b2 = np.asarray(b2, dtype=np.float32)
    ln1_w = np.asarray(ln1_w, dtype=np.float32)
    ln1_b = np.asarray(ln1_b, dtype=np.float32)
    ln2_w = np.asarray(ln2_w, dtype=np.float32)
    ln2_b = np.asarray(ln2_b, dtype=np.float32)

    ln1_trivial = bool(np.all(ln1_w == 1.0) and np.all(ln1_b == 0.0))
    ln2_trivial = bool(np.all(ln2_w == 1.0) and np.all(ln2_b == 0.0))
    b2_zero = bool(np.all(b2 == 0.0))

    key = (ln1_trivial, ln2_trivial, b2_zero)
    if key not in _CACHE:
        _CACHE[key] = _build(*key)
    nc = _CACHE[key]

    wq = _round_fp32r((in_proj_w[:E] * SCALE).T)          # [E, E]
    wk = _round_fp32r(in_proj_w[E:2 * E].T)
    wv = _round_fp32r(in_proj_w[2 * E:].T)
    w1t = _round_fp32r(w1.T)                              # [E, FF]
    w2t = w2.T.astype(np.float16)                         # [FF, E]

    in_maps = []
    for bb in range(B):
        m = {
            "xt": _round_fp32r(x[bb].T),                  # [E, L]
            "wqt": wq, "wkt": wk, "wvt": wv,
            "w1t": w1t, "w2t": w2t, "b1v": b1,
        }
        if not b2_zero:
            m["b2v"] = b2
        if not ln1_trivial:
            m["ln1w"] = ln1_w
            m["ln1b"] = ln1_b
        if not ln2_trivial:
            m["ln2w"] = ln2_w
            m["ln2b"] = ln2_b
        in_maps.append(m)

    res = run_bass_kernel_spmd(nc, in_maps, list(range(B)), trace=_TRACE)
    LAST_RESULT = res
    out = np.stack([np.ascontiguousarray(res.results[bb]["outt"].T)
                    for bb in range(B)])
    return out.astype(np.float32)

